# revision 28
# baseline (speedup 1.0000x reference)
"""Distributed causal MHA for TRN2 (8 NeuronCores), v6: head x batch sharding.

Core c: batch c//2, heads 8*(c%2)..+8 (4 head-pairs). Each core projects
Q/K/V for its 8 heads over all 2048 tokens, runs causal attention, and emits
a PARTIAL out-projection (contraction over its 512 features); the host sums
the two partials per batch and adds the bias.

v6 vs v5: denominators come from a 65th ones-column in V (free: AV matmul
cost is per-rhs-column), so the per-(head,jt) denominator matmuls are gone;
normalization uses ACT Ln -> Exp(scale=-1) (1/den = e^-ln den) instead of
the 8-cycle/elem DVE reciprocal; the odd head's af half is placed by a
SBUF->SBUF DMA partition shift; the jt loop is software-pipelined
(S_h0, exp_h0 || AV_h1(jt-1)+filler, S_h1, exp_h1 || AV_h0(jt)+filler)
with the scores tile single-buffered.
"""

import sys

sys.path.insert(0, "/opt/trn_rl_repo")
import numpy as np
import ml_dtypes
import concourse.bass as bass
import concourse.mybir as mybir
import concourse.tile as tile
from concourse.vector_clock import ScopedClock
from concourse.bass_utils import run_bass_kernel_spmd

B, N, DIM = 4, 2048, 1024
HEADS, DH = 16, 64
INNER = HEADS * DH
SCALE = DH ** -0.5
NEG = -3.0e8
F32 = mybir.dt.float32
BF16 = mybir.dt.bfloat16
AF = mybir.ActivationFunctionType

LAST_RESULT = None


def _drain_and_barrier_patched(self, tick_clock, wait_clock):
    nop_inst = self.nc.sync.nop(nofuse=True)
    wait_clock.add_sem_waits(nop_inst.ins, ScopedClock({None: tick_clock.global_clock}))
    si = nop_inst.ins.sync_info
    waits = list(si.on_wait or []) if si else []
    if len(waits) > 1:
        nop_inst.ins.sync_info = mybir.SyncInfo(
            on_wait=waits[:1], on_update=list(si.on_update or [])
        )
        for i in range(1, len(waits)):
            extra = self.nc.sync.nop(nofuse=True)
            extra.ins.sync_info = mybir.SyncInfo(on_wait=[waits[i]], on_update=[])
    self.nc.sync.drain()
    self.nc.all_engine_barrier()
    popped = self.nc._tile_sem_poison_stack.pop()
    assert popped is self._sem_poison
    self.nc.clear_and_free_semaphores(list(self.sems.allocated().values()))
    self.nc.all_engine_barrier()


tile.TileContext._drain_and_barrier = _drain_and_barrier_patched


def _split_multi_waits(nc):
    for f in nc.m.functions:
        for bb in f.blocks:
            insts = bb.instructions
            if not any(
                i.sync_info and i.sync_info.on_wait and len(i.sync_info.on_wait) > 1
                for i in insts
            ):
                continue
            new = []
            for inst in insts:
                si = inst.sync_info
                waits = list(si.on_wait) if si and si.on_wait else []
                if len(waits) > 1:
                    for w in waits[:-1]:
                        nop = mybir.InstNoOp(
                            name=nc.get_next_instruction_name(), ins=[], outs=[]
                        )
                        nop.engine = inst.engine
                        nop.sync_info = mybir.SyncInfo(on_wait=[w], on_update=[])
                        new.append(nop)
                    inst.sync_info = mybir.SyncInfo(
                        on_wait=[waits[-1]], on_update=list(si.on_update or [])
                    )
                new.append(inst)
            bb.instructions = new


def build_graph():
    nc = bass.Bass("TRN2", target_bir_lowering=False)

    p_xT = nc.declare_dram_parameter("xT", [DIM, N], BF16, isOutput=False)
    p_wq = nc.declare_dram_parameter("w_q", [DIM, 512], BF16, isOutput=False)
    p_wk = nc.declare_dram_parameter("w_k", [DIM, 512], BF16, isOutput=False)
    p_wv = nc.declare_dram_parameter("w_v", [DIM, 512], BF16, isOutput=False)
    p_wo = nc.declare_dram_parameter("w_o", [512, DIM], BF16, isOutput=False)
    p_msk = nc.declare_dram_parameter("maskT", [128, 128], BF16, isOutput=False)
    p_id = nc.declare_dram_parameter("ident", [128, 128], BF16, isOutput=False)
    p_out = nc.declare_dram_parameter("out", [N, DIM], F32, isOutput=True)

    with tile.TileContext(nc) as tc:
        cst = tc.alloc_tile_pool(name="cst", bufs=1)
        xtp = tc.alloc_tile_pool(name="xtp", bufs=1)
        wp = tc.alloc_tile_pool(name="wp", bufs=1)
        kqp = tc.alloc_tile_pool(name="kqp", bufs=1)
        vp = tc.alloc_tile_pool(name="vp", bufs=1)
        afp = tc.alloc_tile_pool(name="afp", bufs=1)
        ewp = tc.alloc_tile_pool(name="ewp", bufs=4)
        rcp = tc.alloc_tile_pool(name="rcp", bufs=2)
        osp = tc.alloc_tile_pool(name="osp", bufs=3)
        ps_sc = tc.alloc_tile_pool(name="ps_sc", bufs=2, space="PSUM")
        ps_av = tc.alloc_tile_pool(name="ps_av", bufs=3, space="PSUM")
        ps_ms = tc.alloc_tile_pool(name="ps_ms", bufs=1, space="PSUM")

        maskT = cst.tile([128, 128], BF16, tag="maskT", name="maskT")
        ident = cst.tile([128, 128], BF16, tag="ident", name="ident")
        oner = cst.tile([128, 64], BF16, tag="oner", name="oner")
        wsrc = cst.tile([1, 8], F32, tag="wsrc", name="wsrc")
        wdst = cst.tile([1, 8], F32, tag="wdst", name="wdst")

        nc.vector.memset(oner[:, :], 1.0)
        nc.vector.memset(wsrc[:, :], 1.0)
        # warm up the exp/ln table-set load while DMAs stream in
        nc.scalar.activation(wdst[:, :], wsrc[:, :], AF.Ln, scale=1.0)
        nc.scalar.activation(wdst[:, :], wsrc[:, :], AF.Exp, scale=-1.0)

        nc.sync.dma_start(maskT[:, :], p_msk[:, :])
        nc.sync.dma_start(ident[:, :], p_id[:, :])

        xt = [xtp.tile([128, N], BF16, tag=f"xt{i}", name=f"xt{i}") for i in range(8)]
        # column-chunked so the first projection chunk can start after ~1MB
        for tc4 in range(4):
            for i in range(8):
                nc.sync.dma_start(
                    xt[i][:, tc4 * 512:(tc4 + 1) * 512],
                    p_xT[i * 128:(i + 1) * 128, tc4 * 512:(tc4 + 1) * 512],
                )
        wq = [wp.tile([128, 512], BF16, tag=f"wq{i}", name=f"wq{i}") for i in range(8)]
        wk = [wp.tile([128, 512], BF16, tag=f"wk{i}", name=f"wk{i}") for i in range(8)]
        wv = [wp.tile([128, 512], BF16, tag=f"wv{i}", name=f"wv{i}") for i in range(8)]
        for i in range(8):
            nc.sync.dma_start(wk[i][:, :], p_wk[i * 128:(i + 1) * 128, :])
            nc.sync.dma_start(wq[i][:, :], p_wq[i * 128:(i + 1) * 128, :])
            nc.sync.dma_start(wv[i][:, :], p_wv[i * 128:(i + 1) * 128, :])
        wo = [wp.tile([128, DIM], BF16, tag=f"wo{i}", name=f"wo{i}") for i in range(4)]
        for i in range(4):
            nc.sync.dma_start(wo[i][:, :], p_wo[i * 128:(i + 1) * 128, :])

        kt = [kqp.tile([128, N], BF16, tag=f"kt{p}", name=f"kt{p}") for p in range(4)]
        qt = [kqp.tile([128, N], BF16, tag=f"qt{p}", name=f"qt{p}") for p in range(4)]
        vT = [kqp.tile([128, N], BF16, tag=f"vT{p}", name=f"vT{p}") for p in range(4)]
        # [tokens, 8 heads x (64 V dims + ones col)]
        vsb = [vp.tile([128, 520], BF16, tag=f"vs{t}", name=f"vs{t}") for t in range(16)]
        for t in range(16):
            nc.vector.memset(
                vsb[t][:, :].rearrange("p (g d) -> p g d", g=8)[:, :, 64:65], 1.0
            )
        af = [afp.tile([128, N], BF16, tag=f"af{p}", name=f"af{p}") for p in range(4)]

        # ------- projection emitters: micro-granular PE filler units ------
        # Each micro-op is ~2 matmuls (or one copy / a few DMAs) so stuffing
        # them into the attention stream never delays the next score matmul
        # by more than ~0.5us.
        def proj_micros(p, tc4, w_tiles, dst_tile):
            cell = {}

            def mm(i):
                def go():
                    if i == 0:
                        cell["ps"] = ps_ms.tile(
                            [128, 512], F32, tag="mm", name=f"pp{p}_{tc4}"
                        )
                    ps = cell["ps"]
                    for k8 in (2 * i, 2 * i + 1):
                        nc.tensor.matmul(
                            ps[:, :],
                            w_tiles[k8][:, p * 128:(p + 1) * 128],
                            xt[k8][:, tc4 * 512:(tc4 + 1) * 512],
                            start=(k8 == 0),
                            stop=(k8 == 7),
                        )
                return go

            def cp():
                nc.vector.tensor_copy(
                    dst_tile[:, tc4 * 512:(tc4 + 1) * 512], cell["ps"][:, :]
                )

            return [mm(0), mm(1), mm(2), mm(3), cp]

        def v_tr(p, tc4):
            # transpose the chunk's 4 feat-major V token-tiles on the PE
            # (bf16 transposes into one PSUM bank; starts at bank
            # granularity are per-region safe), then strided DVE copies
            # into vsb's 65-stride layout.
            cell = {}

            def trs():
                tr = ps_ms.tile([128, 1024], BF16, tag="mm", name=f"vtr{p}_{tc4}")
                cell["tr"] = tr
                for ts in range(4):
                    tt = 4 * tc4 + ts
                    nc.tensor.transpose(
                        tr[:, ts * 128:(ts + 1) * 128],
                        vT[p][:, tt * 128:(tt + 1) * 128],
                        ident[:, :],
                    )

            def cps():
                tr = cell["tr"]
                for ts in range(4):
                    tt = 4 * tc4 + ts
                    dst = vsb[tt][:, 2 * p * 65:(2 * p + 2) * 65].rearrange(
                        "p (g d) -> p g d", g=2
                    )[:, :, 0:64]
                    src = tr[:, ts * 128:(ts + 1) * 128].rearrange(
                        "p (g d) -> p g d", g=2
                    )
                    nc.vector.tensor_copy(dst, src)

            return [trs, cps]

        def proj_unit(p, kind, tc4):
            if kind == "k":
                return proj_micros(p, tc4, wk, kt[p])
            if kind == "q":
                return proj_micros(p, tc4, wq, qt[p])
            return proj_micros(p, tc4, wv, vT[p]) + v_tr(p, tc4)

        PROJ_ORDER = [
            ("k", 0), ("q", 0), ("v", 0), ("q", 1), ("k", 1),
            ("v", 1), ("k", 2), ("q", 2), ("v", 2),
            ("q", 3), ("k", 3), ("v", 3),
        ]

        def make_proj_fillers(p, skip=0):
            out = []
            for kind, tc4 in PROJ_ORDER[skip:]:
                out += proj_unit(p, kind, tc4)
            return out

        def p3_micros(it, oc):
            cell = {}

            def a():
                cell["po"] = ps_ms.tile([128, 512], F32, tag="mm", name=f"po{it}_{oc}")
                for p4 in (0, 1):
                    nc.tensor.matmul(
                        cell["po"][:, :],
                        af[p4][:, it * 128:(it + 1) * 128],
                        wo[p4][:, oc * 512:(oc + 1) * 512],
                        start=(p4 == 0),
                        stop=False,
                    )

            def b():
                for p4 in (2, 3):
                    nc.tensor.matmul(
                        cell["po"][:, :],
                        af[p4][:, it * 128:(it + 1) * 128],
                        wo[p4][:, oc * 512:(oc + 1) * 512],
                        start=False,
                        stop=(p4 == 3),
                    )
                ot = osp.tile([128, 512], F32, tag="os", name=f"os{it}_{oc}")
                nc.vector.tensor_copy(ot[:, :], cell["po"][:, :])
                nc.sync.dma_start(
                    p_out[it * 128:(it + 1) * 128, oc * 512:(oc + 1) * 512],
                    ot[:, :],
                )

            return [a, b]

        # ---------------- attention for one (pair, 512-query window) -----
        def attention(p, qq, af1t, pacer):
            steps = 4 * qq + 4
            av = [
                ps_av.tile([65, 512], F32, tag="av", name=f"av{p}_{qq}_{hi}")
                for hi in (0, 1)
            ]
            qe = (qq + 1) * 512

            def scores(jt):
                # both heads into one [128,1024] tile: h0 bank A, h1 bank B
                # (adjacent row-tiled MMs), then ONE merged exp via a
                # strided 2D-free AP.
                qs = max(jt * 128, qq * 512)
                W = qe - qs
                diag = jt >= qq * 4
                sc = ps_sc.tile([128, 1024], F32, tag="sc", name=f"sc{jt}")
                for hi in (0, 1):
                    off = 64 * hi
                    base = 512 * hi
                    nc.tensor.matmul(
                        sc[:, base:base + W],
                        kt[p][off:off + 64, jt * 128:(jt + 1) * 128],
                        qt[p][off:off + 64, qs:qe],
                        start=True,
                        stop=(not diag),
                    )
                for hi in (0, 1):
                    if diag:
                        nc.tensor.matmul(
                            sc[:, 512 * hi:512 * hi + 128], ident[:, :], maskT[:, :],
                            start=False, stop=True,
                        )
                eW = ewp.tile([128, 1024], BF16, tag="ew", name=f"ew{jt}")
                nc.scalar.activation(
                    eW[:, :].rearrange("p (g w) -> p g w", g=2)[:, :, 0:W],
                    sc[:, :].rearrange("p (g w) -> p g w", g=2)[:, :, 0:W],
                    AF.Exp,
                    scale=SCALE,
                )
                return eW

            def av_accum(jt, eW):
                qs = max(jt * 128, qq * 512)
                qoff = qs - qq * 512
                W = qe - qs
                for hi in (0, 1):
                    h = 2 * p + hi
                    nc.tensor.matmul(
                        av[hi][:, qoff:512],
                        vsb[jt][:, h * 65:(h + 1) * 65],
                        eW[:, 512 * hi:512 * hi + W],
                        start=(jt == 0),
                        stop=(jt == 4 * qq + 3),
                    )

            prev = None
            for jt in range(steps):
                eW = scores(jt)
                if prev is not None:
                    av_accum(jt - 1, prev)
                prev = eW
                pacer.step()
            av_accum(steps - 1, prev)

            # normalization: 1/den = exp(-ln(den)) on ACT, broadcast via a
            # ones-row matmul, one DVE mult per head. Odd head's af half is
            # partition-shifted into place by an SBUF->SBUF DMA.
            for hi in (0, 1):
                lnb = rcp.tile([65, 512], F32, tag="lnb", name=f"lnb{p}_{qq}_{hi}")
                rec = rcp.tile([65, 512], BF16, tag="rec", name=f"rec{p}_{qq}_{hi}")
                nc.scalar.activation(
                    lnb[64:65, :], av[hi][64:65, 0:512], AF.Ln, scale=1.0
                )
                nc.scalar.activation(
                    rec[64:65, :], lnb[64:65, :], AF.Exp, scale=-1.0
                )
                rb = ps_ms.tile([128, 512], F32, tag="mm", name=f"rb{p}_{qq}_{hi}")
                nc.tensor.matmul(
                    rb[0:64, :], oner[64:65, :], rec[64:65, :],
                    start=True, stop=True,
                )
                rbs = rcp.tile([64, 512], F32, tag="rbs", name=f"rbs{p}_{qq}_{hi}")
                nc.vector.tensor_copy(rbs[:, :], rb[0:64, :])
                dst = (
                    af[p][0:64, qq * 512:qe]
                    if hi == 0
                    else af1t[:, qq * 512:qe]
                )
                nc.vector.tensor_mul(dst, av[hi][0:64, 0:512], rbs[:, :])
            nc.sync.dma_start(
                af[p][64:128, qq * 512:qe], af1t[:, qq * 512:qe]
            )

        class Pacer:
            def __init__(self, fillers, total_steps):
                self.fillers = fillers
                self.total = max(1, total_steps)
                self.done = 0
                self.emitted = 0

            def step(self):
                self.done += 1
                want = min(
                    (len(self.fillers) * self.done) // self.total,
                    len(self.fillers),
                )
                while self.emitted < want:
                    self.fillers[self.emitted]()
                    self.emitted += 1

            def drain(self):
                while self.emitted < len(self.fillers):
                    self.fillers[self.emitted]()
                    self.emitted += 1

        # ---------------- main schedule ----------------------------------
        # slim preamble: only the chunks attention(0, qq0) needs; the rest
        # of pair 0's projections pace into its own attention windows.
        for kind, tc4 in PROJ_ORDER[:3]:
            for f in proj_unit(0, kind, tc4):
                f()

        for p in range(4):
            af1t = rcp.tile([64, N], BF16, tag="af1", name=f"af1_{p}")
            if p < 3:
                fillers = (make_proj_fillers(0, skip=3) if p == 0 else []) + \
                    make_proj_fillers(p + 1)
                # front-load pair 0's own remaining projections
                pacer = Pacer(fillers, 28 if p == 0 else 40)
                for qq in range(4):
                    attention(p, qq, af1t, pacer)
                pacer.drain()
            else:
                attention(p, 0, af1t, Pacer([], 4))
                for qq in range(1, 4):
                    u = []
                    for it in range(4 * (qq - 1), 4 * qq):
                        for oc in range(2):
                            u += p3_micros(it, oc)
                    pc = Pacer(u, 4 * qq + 4)
                    attention(p, qq, af1t, pc)
                    pc.drain()

        for it in range(12, 16):
            for oc in range(2):
                for f in p3_micros(it, oc):
                    f()

        for pool in (ps_ms, ps_av, ps_sc, osp, rcp, ewp, afp, vp, kqp, wp, xtp, cst):
            pool.release()

    _split_multi_waits(nc)
    return nc


_GRAPH = None


def _get_graph():
    global _GRAPH
    if _GRAPH is None:
        _GRAPH = build_graph()
    return _GRAPH


def kernel(x, mask, w_qkv, w_out, b_out):
    global LAST_RESULT
    x = np.asarray(x, dtype=np.float32)
    w_qkv = np.asarray(w_qkv, dtype=np.float32)
    w_out = np.asarray(w_out, dtype=np.float32)
    b_out = np.asarray(b_out, dtype=np.float32)

    nc = _get_graph()

    BF = ml_dtypes.bfloat16
    xT = [np.ascontiguousarray(x[b].T.astype(BF)) for b in range(B)]
    ii = np.arange(128)
    maskT = np.where(ii[None, :] >= ii[:, None], 0.0, NEG).astype(BF)
    ident = np.eye(128, dtype=np.float32).astype(BF)

    halves = []
    for h in range(2):
        o = 512 * h
        halves.append(
            {
                "w_q": np.ascontiguousarray(w_qkv[:, o:o + 512].astype(BF)),
                "w_k": np.ascontiguousarray(w_qkv[:, INNER + o:INNER + o + 512].astype(BF)),
                "w_v": np.ascontiguousarray(w_qkv[:, 2 * INNER + o:2 * INNER + o + 512].astype(BF)),
                "w_o": np.ascontiguousarray(w_out[o:o + 512, :].astype(BF)),
            }
        )

    in_maps = []
    for c in range(8):
        b = c // 2
        hv = halves[c % 2]
        in_maps.append(
            {
                "xT": xT[b],
                "w_q": hv["w_q"],
                "w_k": hv["w_k"],
                "w_v": hv["w_v"],
                "w_o": hv["w_o"],
                "maskT": maskT,
                "ident": ident,
            }
        )

    res = run_bass_kernel_spmd(nc, in_maps, list(range(8)))
    LAST_RESULT = res

    out = np.empty((B, N, DIM), dtype=np.float32)
    for b in range(B):
        out[b] = res.results[2 * b]["out"] + res.results[2 * b + 1]["out"] + b_out[None, :]
    return out


# revision 37
# speedup vs baseline: 1.0315x; 1.0315x over previous
"""Distributed causal MHA for TRN2 (8 NeuronCores), v6: head x batch sharding.

Core c: batch c//2, heads 8*(c%2)..+8 (4 head-pairs). Each core projects
Q/K/V for its 8 heads over all 2048 tokens, runs causal attention, and emits
a PARTIAL out-projection (contraction over its 512 features); the host sums
the two partials per batch and adds the bias.

v6 vs v5: denominators come from a 65th ones-column in V (free: AV matmul
cost is per-rhs-column), so the per-(head,jt) denominator matmuls are gone;
normalization uses ACT Ln -> Exp(scale=-1) (1/den = e^-ln den) instead of
the 8-cycle/elem DVE reciprocal; the odd head's af half is placed by a
SBUF->SBUF DMA partition shift; the jt loop is software-pipelined
(S_h0, exp_h0 || AV_h1(jt-1)+filler, S_h1, exp_h1 || AV_h0(jt)+filler)
with the scores tile single-buffered.
"""

import sys

sys.path.insert(0, "/opt/trn_rl_repo")
import numpy as np
import ml_dtypes
import concourse.bass as bass
import concourse.mybir as mybir
import concourse.tile as tile
from concourse.vector_clock import ScopedClock
from concourse.bass_utils import run_bass_kernel_spmd

B, N, DIM = 4, 2048, 1024
HEADS, DH = 16, 64
INNER = HEADS * DH
SCALE = DH ** -0.5
NEG = -3.0e8
F32 = mybir.dt.float32
BF16 = mybir.dt.bfloat16
AF = mybir.ActivationFunctionType

LAST_RESULT = None


def _drain_and_barrier_patched(self, tick_clock, wait_clock):
    nop_inst = self.nc.sync.nop(nofuse=True)
    wait_clock.add_sem_waits(nop_inst.ins, ScopedClock({None: tick_clock.global_clock}))
    si = nop_inst.ins.sync_info
    waits = list(si.on_wait or []) if si else []
    if len(waits) > 1:
        nop_inst.ins.sync_info = mybir.SyncInfo(
            on_wait=waits[:1], on_update=list(si.on_update or [])
        )
        for i in range(1, len(waits)):
            extra = self.nc.sync.nop(nofuse=True)
            extra.ins.sync_info = mybir.SyncInfo(on_wait=[waits[i]], on_update=[])
    self.nc.sync.drain()
    self.nc.all_engine_barrier()
    popped = self.nc._tile_sem_poison_stack.pop()
    assert popped is self._sem_poison
    self.nc.clear_and_free_semaphores(list(self.sems.allocated().values()))
    self.nc.all_engine_barrier()


tile.TileContext._drain_and_barrier = _drain_and_barrier_patched


def _split_multi_waits(nc):
    for f in nc.m.functions:
        for bb in f.blocks:
            insts = bb.instructions
            if not any(
                i.sync_info and i.sync_info.on_wait and len(i.sync_info.on_wait) > 1
                for i in insts
            ):
                continue
            new = []
            for inst in insts:
                si = inst.sync_info
                waits = list(si.on_wait) if si and si.on_wait else []
                if len(waits) > 1:
                    for w in waits[:-1]:
                        nop = mybir.InstNoOp(
                            name=nc.get_next_instruction_name(), ins=[], outs=[]
                        )
                        nop.engine = inst.engine
                        nop.sync_info = mybir.SyncInfo(on_wait=[w], on_update=[])
                        new.append(nop)
                    inst.sync_info = mybir.SyncInfo(
                        on_wait=[waits[-1]], on_update=list(si.on_update or [])
                    )
                new.append(inst)
            bb.instructions = new


def build_graph():
    nc = bass.Bass("TRN2", target_bir_lowering=False)

    p_xT = nc.declare_dram_parameter("xT", [DIM, N], BF16, isOutput=False)
    p_wq = nc.declare_dram_parameter("w_q", [DIM, 512], BF16, isOutput=False)
    p_wk = nc.declare_dram_parameter("w_k", [DIM, 512], BF16, isOutput=False)
    p_wv = nc.declare_dram_parameter("w_v", [DIM, 512], BF16, isOutput=False)
    p_wo = nc.declare_dram_parameter("w_o", [512, DIM], BF16, isOutput=False)
    p_msk = nc.declare_dram_parameter("mask01", [128, 128], BF16, isOutput=False)
    p_id = nc.declare_dram_parameter("ident", [128, 128], BF16, isOutput=False)
    p_out = nc.declare_dram_parameter("out", [N, DIM], BF16, isOutput=True)

    with tile.TileContext(nc) as tc:
        cst = tc.alloc_tile_pool(name="cst", bufs=1)
        xtp = tc.alloc_tile_pool(name="xtp", bufs=1)
        wp = tc.alloc_tile_pool(name="wp", bufs=1)
        kqp = tc.alloc_tile_pool(name="kqp", bufs=1)
        vp = tc.alloc_tile_pool(name="vp", bufs=1)
        afp = tc.alloc_tile_pool(name="afp", bufs=1)
        ewp = tc.alloc_tile_pool(name="ewp", bufs=4)
        rcp = tc.alloc_tile_pool(name="rcp", bufs=2)
        osp = tc.alloc_tile_pool(name="osp", bufs=3)
        ps_sc = tc.alloc_tile_pool(name="ps_sc", bufs=2, space="PSUM")
        ps_av = tc.alloc_tile_pool(name="ps_av", bufs=3, space="PSUM")
        ps_ms = tc.alloc_tile_pool(name="ps_ms", bufs=1, space="PSUM")

        mask01 = cst.tile([128, 128], BF16, tag="mask01", name="mask01")
        ident = cst.tile([128, 128], BF16, tag="ident", name="ident")
        oner = cst.tile([128, 64], BF16, tag="oner", name="oner")
        wsrc = cst.tile([1, 8], F32, tag="wsrc", name="wsrc")
        wdst = cst.tile([1, 8], F32, tag="wdst", name="wdst")

        nc.vector.memset(oner[:, :], 1.0)
        nc.vector.memset(wsrc[:, :], 1.0)
        # warm up the exp/ln table-set load while DMAs stream in
        nc.scalar.activation(wdst[:, :], wsrc[:, :], AF.Ln, scale=1.0)
        nc.scalar.activation(wdst[:, :], wsrc[:, :], AF.Exp, scale=-1.0)

        nc.sync.dma_start(mask01[:, :], p_msk[:, :])
        nc.sync.dma_start(ident[:, :], p_id[:, :])

        xt = [xtp.tile([128, N], BF16, tag=f"xt{i}", name=f"xt{i}") for i in range(8)]
        wq = [wp.tile([128, 512], BF16, tag=f"wq{i}", name=f"wq{i}") for i in range(8)]
        wk = [wp.tile([128, 512], BF16, tag=f"wk{i}", name=f"wk{i}") for i in range(8)]
        wv = [wp.tile([128, 512], BF16, tag=f"wv{i}", name=f"wv{i}") for i in range(8)]
        wo = [wp.tile([128, DIM], BF16, tag=f"wo{i}", name=f"wo{i}") for i in range(4)]

        def _xt_chunk(tc4):
            for i in range(8):
                nc.sync.dma_start(
                    xt[i][:, tc4 * 512:(tc4 + 1) * 512],
                    p_xT[i * 128:(i + 1) * 128, tc4 * 512:(tc4 + 1) * 512],
                )

        # interleave weight loads with xT column chunks so the first
        # projection chunk (needs wk + xT chunk 0) starts after ~2MB.
        for i in range(8):
            nc.sync.dma_start(wk[i][:, :], p_wk[i * 128:(i + 1) * 128, :])
        _xt_chunk(0)
        for i in range(8):
            nc.sync.dma_start(wq[i][:, :], p_wq[i * 128:(i + 1) * 128, :])
        _xt_chunk(1)
        for i in range(8):
            nc.sync.dma_start(wv[i][:, :], p_wv[i * 128:(i + 1) * 128, :])
        _xt_chunk(2)
        for i in range(4):
            nc.sync.dma_start(wo[i][:, :], p_wo[i * 128:(i + 1) * 128, :])
        _xt_chunk(3)

        kt = [kqp.tile([128, N], BF16, tag=f"kt{p}", name=f"kt{p}") for p in range(4)]
        qt = [kqp.tile([128, N], BF16, tag=f"qt{p}", name=f"qt{p}") for p in range(4)]
        vT = [kqp.tile([128, N], BF16, tag=f"vT{p}", name=f"vT{p}") for p in range(4)]
        # [tokens, 8 heads x (64 V dims + ones col)]
        vsb = [vp.tile([128, 520], BF16, tag=f"vs{t}", name=f"vs{t}") for t in range(16)]
        for t in range(16):
            nc.vector.memset(
                vsb[t][:, :].rearrange("p (g d) -> p g d", g=8)[:, :, 64:65], 1.0
            )
        af = [afp.tile([128, N], BF16, tag=f"af{p}", name=f"af{p}") for p in range(4)]

        # ------- projection emitters: micro-granular PE filler units ------
        # Each micro-op is ~2 matmuls (or one copy / a few DMAs) so stuffing
        # them into the attention stream never delays the next score matmul
        # by more than ~0.5us.
        def proj_micros(p, tc4, w_tiles, dst_tile):
            cell = {}

            def mm(i):
                def go():
                    if i == 0:
                        cell["ps"] = ps_ms.tile(
                            [128, 512], F32, tag="mm", name=f"pp{p}_{tc4}"
                        )
                    ps = cell["ps"]
                    for k8 in (2 * i, 2 * i + 1):
                        nc.tensor.matmul(
                            ps[:, :],
                            w_tiles[k8][:, p * 128:(p + 1) * 128],
                            xt[k8][:, tc4 * 512:(tc4 + 1) * 512],
                            start=(k8 == 0),
                            stop=(k8 == 7),
                        )
                return go

            def cp():
                nc.vector.tensor_copy(
                    dst_tile[:, tc4 * 512:(tc4 + 1) * 512], cell["ps"][:, :]
                )

            return [mm(0), mm(1), mm(2), mm(3), cp]

        def v_tr(p, tc4):
            # transpose the chunk's 4 feat-major V token-tiles on the PE
            # (bf16 transposes into one PSUM bank; starts at bank
            # granularity are per-region safe), then strided DVE copies
            # into vsb's 65-stride layout.
            cell = {}

            def trs():
                tr = ps_ms.tile([128, 1024], BF16, tag="mm", name=f"vtr{p}_{tc4}")
                cell["tr"] = tr
                for ts in range(4):
                    tt = 4 * tc4 + ts
                    nc.tensor.transpose(
                        tr[:, ts * 128:(ts + 1) * 128],
                        vT[p][:, tt * 128:(tt + 1) * 128],
                        ident[:, :],
                    )

            def cps():
                tr = cell["tr"]
                for ts in range(4):
                    tt = 4 * tc4 + ts
                    dst = vsb[tt][:, 2 * p * 65:(2 * p + 2) * 65].rearrange(
                        "p (g d) -> p g d", g=2
                    )[:, :, 0:64]
                    src = tr[:, ts * 128:(ts + 1) * 128].rearrange(
                        "p (g d) -> p g d", g=2
                    )
                    nc.vector.tensor_copy(dst, src)

            return [trs, cps]

        def proj_unit(p, kind, tc4):
            if kind == "k":
                return proj_micros(p, tc4, wk, kt[p])
            if kind == "q":
                return proj_micros(p, tc4, wq, qt[p])
            return proj_micros(p, tc4, wv, vT[p]) + v_tr(p, tc4)

        PROJ_ORDER = [
            ("k", 0), ("q", 0), ("v", 0), ("q", 1), ("k", 1),
            ("v", 1), ("k", 2), ("q", 2), ("v", 2),
            ("q", 3), ("k", 3), ("v", 3),
        ]

        def make_proj_fillers(p, skip=0):
            out = []
            for kind, tc4 in PROJ_ORDER[skip:]:
                out += proj_unit(p, kind, tc4)
            return out

        def p3_micros(it, oc):
            cell = {}

            def a():
                cell["po"] = ps_ms.tile([128, 512], F32, tag="mm", name=f"po{it}_{oc}")
                for p4 in (0, 1):
                    nc.tensor.matmul(
                        cell["po"][:, :],
                        af[p4][:, it * 128:(it + 1) * 128],
                        wo[p4][:, oc * 512:(oc + 1) * 512],
                        start=(p4 == 0),
                        stop=False,
                    )

            def b():
                for p4 in (2, 3):
                    nc.tensor.matmul(
                        cell["po"][:, :],
                        af[p4][:, it * 128:(it + 1) * 128],
                        wo[p4][:, oc * 512:(oc + 1) * 512],
                        start=False,
                        stop=(p4 == 3),
                    )
                ot = osp.tile([128, 512], BF16, tag="os", name=f"os{it}_{oc}")
                nc.vector.tensor_copy(ot[:, :], cell["po"][:, :])
                nc.sync.dma_start(
                    p_out[it * 128:(it + 1) * 128, oc * 512:(oc + 1) * 512],
                    ot[:, :],
                )

            return [a, b]

        # ---------------- attention for one (pair, 512-query window) -----
        def attention(p, qq, af1t, pacer):
            steps = 4 * qq + 4
            av = [
                ps_av.tile([65, 512], F32, tag="av", name=f"av{p}_{qq}_{hi}")
                for hi in (0, 1)
            ]
            qe = (qq + 1) * 512

            def scores(jt):
                # both heads into one [128,1024] tile: h0 bank A, h1 bank B
                # (adjacent row-tiled MMs), then ONE merged exp via a
                # strided 2D-free AP. The causal diagonal is zeroed on the
                # idle GpSimd engine after the exp (multiplicative 0/1
                # mask), which also zeroes its denominator contribution.
                qs = max(jt * 128, qq * 512)
                W = qe - qs
                diag = jt >= qq * 4
                sc = ps_sc.tile([128, 1024], F32, tag="sc", name=f"sc{jt}")
                for hi in (0, 1):
                    off = 64 * hi
                    base = 512 * hi
                    nc.tensor.matmul(
                        sc[:, base:base + W],
                        kt[p][off:off + 64, jt * 128:(jt + 1) * 128],
                        qt[p][off:off + 64, qs:qe],
                        start=True,
                        stop=True,
                    )
                eW = ewp.tile([128, 1024], BF16, tag="ew", name=f"ew{jt}")
                nc.scalar.activation(
                    eW[:, :].rearrange("p (g w) -> p g w", g=2)[:, :, 0:W],
                    sc[:, :].rearrange("p (g w) -> p g w", g=2)[:, :, 0:W],
                    AF.Exp,
                    scale=SCALE,
                )
                if diag:
                    for hi in (0, 1):
                        ds = eW[:, 512 * hi:512 * hi + 128]
                        nc.gpsimd.tensor_mul(ds, ds, mask01[:, :])
                return eW

            def av_accum(jt, eW):
                qs = max(jt * 128, qq * 512)
                qoff = qs - qq * 512
                W = qe - qs
                for hi in (0, 1):
                    h = 2 * p + hi
                    nc.tensor.matmul(
                        av[hi][:, qoff:512],
                        vsb[jt][:, h * 65:(h + 1) * 65],
                        eW[:, 512 * hi:512 * hi + W],
                        start=(jt == 0),
                        stop=(jt == 4 * qq + 3),
                    )

            prev = None
            for jt in range(steps):
                eW = scores(jt)
                if prev is not None:
                    av_accum(jt - 1, prev)
                prev = eW
                pacer.step()
            av_accum(steps - 1, prev)

            # normalization: 1/den = exp(-ln(den)) on ACT, broadcast via a
            # ones-row matmul, one DVE mult per head. Odd head's af half is
            # partition-shifted into place by an SBUF->SBUF DMA.
            for hi in (0, 1):
                lnb = rcp.tile([65, 512], F32, tag="lnb", name=f"lnb{p}_{qq}_{hi}")
                rec = rcp.tile([65, 512], BF16, tag="rec", name=f"rec{p}_{qq}_{hi}")
                nc.scalar.activation(
                    lnb[64:65, :], av[hi][64:65, 0:512], AF.Ln, scale=1.0
                )
                nc.scalar.activation(
                    rec[64:65, :], lnb[64:65, :], AF.Exp, scale=-1.0
                )
                rb = ps_ms.tile([128, 512], F32, tag="mm", name=f"rb{p}_{qq}_{hi}")
                nc.tensor.matmul(
                    rb[0:64, :], oner[64:65, :], rec[64:65, :],
                    start=True, stop=True,
                )
                rbs = rcp.tile([64, 512], F32, tag="rbs", name=f"rbs{p}_{qq}_{hi}")
                nc.vector.tensor_copy(rbs[:, :], rb[0:64, :])
                dst = (
                    af[p][0:64, qq * 512:qe]
                    if hi == 0
                    else af1t[:, qq * 512:qe]
                )
                nc.vector.tensor_mul(dst, av[hi][0:64, 0:512], rbs[:, :])
            nc.sync.dma_start(
                af[p][64:128, qq * 512:qe], af1t[:, qq * 512:qe]
            )

        class Pacer:
            def __init__(self, fillers, total_steps):
                self.fillers = fillers
                self.total = max(1, total_steps)
                self.done = 0
                self.emitted = 0

            def step(self):
                self.done += 1
                want = min(
                    (len(self.fillers) * self.done) // self.total,
                    len(self.fillers),
                )
                while self.emitted < want:
                    self.fillers[self.emitted]()
                    self.emitted += 1

            def drain(self):
                while self.emitted < len(self.fillers):
                    self.fillers[self.emitted]()
                    self.emitted += 1

        # ---------------- main schedule ----------------------------------
        # slim preamble: only the chunks attention(0, qq0) needs; the rest
        # of pair 0's projections pace into its own attention windows.
        for kind, tc4 in PROJ_ORDER[:3]:
            for f in proj_unit(0, kind, tc4):
                f()

        for p in range(4):
            af1t = rcp.tile([64, N], BF16, tag="af1", name=f"af1_{p}")
            if p < 3:
                fillers = (make_proj_fillers(0, skip=3) if p == 0 else []) + \
                    make_proj_fillers(p + 1)
                # front-load pair 0's own remaining projections
                pacer = Pacer(fillers, 28 if p == 0 else 40)
                for qq in range(4):
                    attention(p, qq, af1t, pacer)
                pacer.drain()
            else:
                attention(p, 0, af1t, Pacer([], 4))
                for qq in range(1, 4):
                    u = []
                    for it in range(4 * (qq - 1), 4 * qq):
                        for oc in range(2):
                            u += p3_micros(it, oc)
                    pc = Pacer(u, 4 * qq + 4)
                    attention(p, qq, af1t, pc)
                    pc.drain()

        for it in range(12, 16):
            for oc in range(2):
                for f in p3_micros(it, oc):
                    f()

        for pool in (ps_ms, ps_av, ps_sc, osp, rcp, ewp, afp, vp, kqp, wp, xtp, cst):
            pool.release()

    _split_multi_waits(nc)
    return nc


_GRAPH = None


def _get_graph():
    global _GRAPH
    if _GRAPH is None:
        _GRAPH = build_graph()
    return _GRAPH


def kernel(x, mask, w_qkv, w_out, b_out):
    global LAST_RESULT
    x = np.asarray(x, dtype=np.float32)
    w_qkv = np.asarray(w_qkv, dtype=np.float32)
    w_out = np.asarray(w_out, dtype=np.float32)
    b_out = np.asarray(b_out, dtype=np.float32)

    nc = _get_graph()

    BF = ml_dtypes.bfloat16
    xT = [np.ascontiguousarray(x[b].T.astype(BF)) for b in range(B)]
    ii = np.arange(128)
    mask01 = np.where(ii[None, :] >= ii[:, None], 1.0, 0.0).astype(BF)
    ident = np.eye(128, dtype=np.float32).astype(BF)

    halves = []
    for h in range(2):
        o = 512 * h
        halves.append(
            {
                "w_q": np.ascontiguousarray(w_qkv[:, o:o + 512].astype(BF)),
                "w_k": np.ascontiguousarray(w_qkv[:, INNER + o:INNER + o + 512].astype(BF)),
                "w_v": np.ascontiguousarray(w_qkv[:, 2 * INNER + o:2 * INNER + o + 512].astype(BF)),
                "w_o": np.ascontiguousarray(w_out[o:o + 512, :].astype(BF)),
            }
        )

    in_maps = []
    for c in range(8):
        b = c // 2
        hv = halves[c % 2]
        in_maps.append(
            {
                "xT": xT[b],
                "w_q": hv["w_q"],
                "w_k": hv["w_k"],
                "w_v": hv["w_v"],
                "w_o": hv["w_o"],
                "mask01": mask01,
                "ident": ident,
            }
        )

    res = run_bass_kernel_spmd(nc, in_maps, list(range(8)))
    LAST_RESULT = res

    out = np.empty((B, N, DIM), dtype=np.float32)
    for b in range(B):
        out[b] = (
            res.results[2 * b]["out"].astype(np.float32)
            + res.results[2 * b + 1]["out"].astype(np.float32)
            + b_out[None, :]
        )
    return out


# revision 38
# speedup vs baseline: 1.0529x; 1.0207x over previous
"""Distributed causal MHA for TRN2 (8 NeuronCores), v6: head x batch sharding.

Core c: batch c//2, heads 8*(c%2)..+8 (4 head-pairs). Each core projects
Q/K/V for its 8 heads over all 2048 tokens, runs causal attention, and emits
a PARTIAL out-projection (contraction over its 512 features); the host sums
the two partials per batch and adds the bias.

v6 vs v5: denominators come from a 65th ones-column in V (free: AV matmul
cost is per-rhs-column), so the per-(head,jt) denominator matmuls are gone;
normalization uses ACT Ln -> Exp(scale=-1) (1/den = e^-ln den) instead of
the 8-cycle/elem DVE reciprocal; the odd head's af half is placed by a
SBUF->SBUF DMA partition shift; the jt loop is software-pipelined
(S_h0, exp_h0 || AV_h1(jt-1)+filler, S_h1, exp_h1 || AV_h0(jt)+filler)
with the scores tile single-buffered.
"""

import sys

sys.path.insert(0, "/opt/trn_rl_repo")
import numpy as np
import ml_dtypes
import concourse.bass as bass
import concourse.mybir as mybir
import concourse.tile as tile
from concourse.vector_clock import ScopedClock
from concourse.bass_utils import run_bass_kernel_spmd

B, N, DIM = 4, 2048, 1024
HEADS, DH = 16, 64
INNER = HEADS * DH
SCALE = DH ** -0.5
NEG = -3.0e8
F32 = mybir.dt.float32
BF16 = mybir.dt.bfloat16
AF = mybir.ActivationFunctionType

LAST_RESULT = None


def _drain_and_barrier_patched(self, tick_clock, wait_clock):
    nop_inst = self.nc.sync.nop(nofuse=True)
    wait_clock.add_sem_waits(nop_inst.ins, ScopedClock({None: tick_clock.global_clock}))
    si = nop_inst.ins.sync_info
    waits = list(si.on_wait or []) if si else []
    if len(waits) > 1:
        nop_inst.ins.sync_info = mybir.SyncInfo(
            on_wait=waits[:1], on_update=list(si.on_update or [])
        )
        for i in range(1, len(waits)):
            extra = self.nc.sync.nop(nofuse=True)
            extra.ins.sync_info = mybir.SyncInfo(on_wait=[waits[i]], on_update=[])
    self.nc.sync.drain()
    self.nc.all_engine_barrier()
    popped = self.nc._tile_sem_poison_stack.pop()
    assert popped is self._sem_poison
    self.nc.clear_and_free_semaphores(list(self.sems.allocated().values()))
    self.nc.all_engine_barrier()


tile.TileContext._drain_and_barrier = _drain_and_barrier_patched


def _split_multi_waits(nc):
    for f in nc.m.functions:
        for bb in f.blocks:
            insts = bb.instructions
            if not any(
                i.sync_info and i.sync_info.on_wait and len(i.sync_info.on_wait) > 1
                for i in insts
            ):
                continue
            new = []
            for inst in insts:
                si = inst.sync_info
                waits = list(si.on_wait) if si and si.on_wait else []
                if len(waits) > 1:
                    for w in waits[:-1]:
                        nop = mybir.InstNoOp(
                            name=nc.get_next_instruction_name(), ins=[], outs=[]
                        )
                        nop.engine = inst.engine
                        nop.sync_info = mybir.SyncInfo(on_wait=[w], on_update=[])
                        new.append(nop)
                    inst.sync_info = mybir.SyncInfo(
                        on_wait=[waits[-1]], on_update=list(si.on_update or [])
                    )
                new.append(inst)
            bb.instructions = new


def build_graph():
    nc = bass.Bass("TRN2", target_bir_lowering=False)

    p_xT = nc.declare_dram_parameter("xT", [DIM, N], BF16, isOutput=False)
    p_wq = nc.declare_dram_parameter("w_q", [DIM, 512], BF16, isOutput=False)
    p_wk = nc.declare_dram_parameter("w_k", [DIM, 512], BF16, isOutput=False)
    p_wv = nc.declare_dram_parameter("w_v", [DIM, 512], BF16, isOutput=False)
    p_wo = nc.declare_dram_parameter("w_o", [512, DIM], BF16, isOutput=False)
    p_msk = nc.declare_dram_parameter("mask01", [128, 128], BF16, isOutput=False)
    p_id = nc.declare_dram_parameter("ident", [128, 128], BF16, isOutput=False)
    p_out = nc.declare_dram_parameter("out", [N, DIM], BF16, isOutput=True)

    with tile.TileContext(nc) as tc:
        cst = tc.alloc_tile_pool(name="cst", bufs=1)
        xtp = tc.alloc_tile_pool(name="xtp", bufs=1)
        wp = tc.alloc_tile_pool(name="wp", bufs=1)
        kqp = tc.alloc_tile_pool(name="kqp", bufs=1)
        vp = tc.alloc_tile_pool(name="vp", bufs=1)
        afp = tc.alloc_tile_pool(name="afp", bufs=1)
        ewp = tc.alloc_tile_pool(name="ewp", bufs=4)
        rcp = tc.alloc_tile_pool(name="rcp", bufs=2)
        osp = tc.alloc_tile_pool(name="osp", bufs=3)
        ps_sc = tc.alloc_tile_pool(name="ps_sc", bufs=2, space="PSUM")
        ps_av = tc.alloc_tile_pool(name="ps_av", bufs=3, space="PSUM")
        ps_ms = tc.alloc_tile_pool(name="ps_ms", bufs=1, space="PSUM")

        mask01 = cst.tile([128, 128], BF16, tag="mask01", name="mask01")
        ident = cst.tile([128, 128], BF16, tag="ident", name="ident")
        oner = cst.tile([128, 64], BF16, tag="oner", name="oner")
        wsrc = cst.tile([1, 8], F32, tag="wsrc", name="wsrc")
        wdst = cst.tile([1, 8], F32, tag="wdst", name="wdst")

        nc.vector.memset(oner[:, :], 1.0)
        nc.vector.memset(wsrc[:, :], 1.0)
        # warm up the exp/ln table-set load while DMAs stream in
        nc.scalar.activation(wdst[:, :], wsrc[:, :], AF.Ln, scale=1.0)
        nc.scalar.activation(wdst[:, :], wsrc[:, :], AF.Exp, scale=-1.0)

        nc.sync.dma_start(mask01[:, :], p_msk[:, :])
        nc.sync.dma_start(ident[:, :], p_id[:, :])

        xt = [xtp.tile([128, N], BF16, tag=f"xt{i}", name=f"xt{i}") for i in range(8)]
        wq = [wp.tile([128, 512], BF16, tag=f"wq{i}", name=f"wq{i}") for i in range(8)]
        wk = [wp.tile([128, 512], BF16, tag=f"wk{i}", name=f"wk{i}") for i in range(8)]
        wv = [wp.tile([128, 512], BF16, tag=f"wv{i}", name=f"wv{i}") for i in range(8)]
        wo = [wp.tile([128, DIM], BF16, tag=f"wo{i}", name=f"wo{i}") for i in range(4)]

        def _xt_chunk(tc4):
            for i in range(8):
                nc.sync.dma_start(
                    xt[i][:, tc4 * 512:(tc4 + 1) * 512],
                    p_xT[i * 128:(i + 1) * 128, tc4 * 512:(tc4 + 1) * 512],
                )

        # interleave weight loads with xT column chunks so the first
        # projection chunk (needs wk + xT chunk 0) starts after ~2MB.
        for i in range(8):
            nc.sync.dma_start(wk[i][:, :], p_wk[i * 128:(i + 1) * 128, :])
        _xt_chunk(0)
        for i in range(8):
            nc.sync.dma_start(wq[i][:, :], p_wq[i * 128:(i + 1) * 128, :])
        _xt_chunk(1)
        for i in range(8):
            nc.sync.dma_start(wv[i][:, :], p_wv[i * 128:(i + 1) * 128, :])
        _xt_chunk(2)
        for i in range(4):
            nc.sync.dma_start(wo[i][:, :], p_wo[i * 128:(i + 1) * 128, :])
        _xt_chunk(3)

        kt = [kqp.tile([128, N], BF16, tag=f"kt{p}", name=f"kt{p}") for p in range(4)]
        qt = [kqp.tile([128, N], BF16, tag=f"qt{p}", name=f"qt{p}") for p in range(4)]
        vT = [kqp.tile([128, N], BF16, tag=f"vT{p}", name=f"vT{p}") for p in range(4)]
        # [tokens, 8 heads x (64 V dims + ones col)]
        vsb = [vp.tile([128, 520], BF16, tag=f"vs{t}", name=f"vs{t}") for t in range(16)]
        for t in range(16):
            nc.vector.memset(
                vsb[t][:, :].rearrange("p (g d) -> p g d", g=8)[:, :, 64:65], 1.0
            )
        af = [afp.tile([128, N], BF16, tag=f"af{p}", name=f"af{p}") for p in range(4)]

        # ------- projection emitters: micro-granular PE filler units ------
        # Each micro-op is ~2 matmuls (or one copy / a few DMAs) so stuffing
        # them into the attention stream never delays the next score matmul
        # by more than ~0.5us.
        def proj_micros(p, tc4, w_tiles, dst_tile):
            cell = {}

            def mm(i):
                def go():
                    if i == 0:
                        cell["ps"] = ps_ms.tile(
                            [128, 512], F32, tag="mm", name=f"pp{p}_{tc4}"
                        )
                    ps = cell["ps"]
                    for k8 in (2 * i, 2 * i + 1):
                        nc.tensor.matmul(
                            ps[:, :],
                            w_tiles[k8][:, p * 128:(p + 1) * 128],
                            xt[k8][:, tc4 * 512:(tc4 + 1) * 512],
                            start=(k8 == 0),
                            stop=(k8 == 7),
                        )
                return go

            def cp():
                nc.vector.tensor_copy(
                    dst_tile[:, tc4 * 512:(tc4 + 1) * 512], cell["ps"][:, :]
                )

            return [mm(0), mm(1), mm(2), mm(3), cp]

        def v_tr(p, tc4):
            # transpose the chunk's 4 feat-major V token-tiles on the PE
            # (bf16 transposes into one PSUM bank; starts at bank
            # granularity are per-region safe), then strided DVE copies
            # into vsb's 65-stride layout.
            cell = {}

            def trs():
                tr = ps_ms.tile([128, 1024], BF16, tag="mm", name=f"vtr{p}_{tc4}")
                cell["tr"] = tr
                for ts in range(4):
                    tt = 4 * tc4 + ts
                    nc.tensor.transpose(
                        tr[:, ts * 128:(ts + 1) * 128],
                        vT[p][:, tt * 128:(tt + 1) * 128],
                        ident[:, :],
                    )

            def cps():
                tr = cell["tr"]
                for ts in range(4):
                    tt = 4 * tc4 + ts
                    dst = vsb[tt][:, 2 * p * 65:(2 * p + 2) * 65].rearrange(
                        "p (g d) -> p g d", g=2
                    )[:, :, 0:64]
                    src = tr[:, ts * 128:(ts + 1) * 128].rearrange(
                        "p (g d) -> p g d", g=2
                    )
                    nc.vector.tensor_copy(dst, src)

            return [trs, cps]

        def proj_unit(p, kind, tc4):
            if kind == "k":
                return proj_micros(p, tc4, wk, kt[p])
            if kind == "q":
                return proj_micros(p, tc4, wq, qt[p])
            return proj_micros(p, tc4, wv, vT[p]) + v_tr(p, tc4)

        PROJ_ORDER = [
            ("k", 0), ("q", 0), ("v", 0), ("q", 1), ("k", 1),
            ("v", 1), ("k", 2), ("q", 2), ("v", 2),
            ("q", 3), ("k", 3), ("v", 3),
        ]

        def make_proj_fillers(p, skip=0):
            out = []
            for kind, tc4 in PROJ_ORDER[skip:]:
                out += proj_unit(p, kind, tc4)
            return out

        def p3_micros(it, oc):
            cell = {}

            def a():
                cell["po"] = ps_ms.tile([128, 512], F32, tag="mm", name=f"po{it}_{oc}")
                for p4 in (0, 1):
                    nc.tensor.matmul(
                        cell["po"][:, :],
                        af[p4][:, it * 128:(it + 1) * 128],
                        wo[p4][:, oc * 512:(oc + 1) * 512],
                        start=(p4 == 0),
                        stop=False,
                    )

            def b():
                for p4 in (2, 3):
                    nc.tensor.matmul(
                        cell["po"][:, :],
                        af[p4][:, it * 128:(it + 1) * 128],
                        wo[p4][:, oc * 512:(oc + 1) * 512],
                        start=False,
                        stop=(p4 == 3),
                    )
                ot = osp.tile([128, 512], BF16, tag="os", name=f"os{it}_{oc}")
                nc.vector.tensor_copy(ot[:, :], cell["po"][:, :])
                nc.sync.dma_start(
                    p_out[it * 128:(it + 1) * 128, oc * 512:(oc + 1) * 512],
                    ot[:, :],
                )

            return [a, b]

        # ---------------- attention for one (pair, 512-query window) -----
        def attention(p, qq, af1t, pacer):
            steps = 4 * qq + 4
            av = [
                ps_av.tile([65, 512], F32, tag="av", name=f"av{p}_{qq}_{hi}")
                for hi in (0, 1)
            ]
            qe = (qq + 1) * 512

            def scores(jt):
                # both heads into one [128,1024] tile: h0 bank A, h1 bank B
                # (adjacent row-tiled MMs), then ONE merged exp via a
                # strided 2D-free AP. The causal diagonal is zeroed on the
                # idle GpSimd engine after the exp (multiplicative 0/1
                # mask), which also zeroes its denominator contribution.
                qs = max(jt * 128, qq * 512)
                W = qe - qs
                diag = jt >= qq * 4
                sc = ps_sc.tile([128, 1024], F32, tag="sc", name=f"sc{jt}")
                for hi in (0, 1):
                    off = 64 * hi
                    base = 512 * hi
                    nc.tensor.matmul(
                        sc[:, base:base + W],
                        kt[p][off:off + 64, jt * 128:(jt + 1) * 128],
                        qt[p][off:off + 64, qs:qe],
                        start=True,
                        stop=True,
                    )
                eW = ewp.tile([128, 1024], BF16, tag="ew", name=f"ew{jt}")
                nc.scalar.activation(
                    eW[:, :].rearrange("p (g w) -> p g w", g=2)[:, :, 0:W],
                    sc[:, :].rearrange("p (g w) -> p g w", g=2)[:, :, 0:W],
                    AF.Exp,
                    scale=SCALE,
                )
                if diag:
                    for hi in (0, 1):
                        ds = eW[:, 512 * hi:512 * hi + 128]
                        nc.gpsimd.tensor_mul(ds, ds, mask01[:, :])
                return eW

            def av_accum(jt, eW):
                qs = max(jt * 128, qq * 512)
                qoff = qs - qq * 512
                W = qe - qs
                for hi in (0, 1):
                    h = 2 * p + hi
                    nc.tensor.matmul(
                        av[hi][:, qoff:512],
                        vsb[jt][:, h * 65:(h + 1) * 65],
                        eW[:, 512 * hi:512 * hi + W],
                        start=(jt == 0),
                        stop=(jt == 4 * qq + 3),
                    )

            # depth-2 software pipeline: AV consumes the exp output from two
            # steps back, so no PE instruction waits on a fresh dependency
            # (keeps the PE dense enough that the HAM clock stays at 2.4GHz)
            pend = []
            for jt in range(steps):
                eW = scores(jt)
                if len(pend) == 2:
                    av_accum(*pend.pop(0))
                pend.append((jt, eW))
                pacer.step()
            for jt_, eW_ in pend:
                av_accum(jt_, eW_)

            # normalization: 1/den = exp(-ln(den)) on ACT, broadcast via a
            # ones-row matmul, one DVE mult per head. Odd head's af half is
            # partition-shifted into place by an SBUF->SBUF DMA.
            for hi in (0, 1):
                lnb = rcp.tile([65, 512], F32, tag="lnb", name=f"lnb{p}_{qq}_{hi}")
                rec = rcp.tile([65, 512], BF16, tag="rec", name=f"rec{p}_{qq}_{hi}")
                nc.scalar.activation(
                    lnb[64:65, :], av[hi][64:65, 0:512], AF.Ln, scale=1.0
                )
                nc.scalar.activation(
                    rec[64:65, :], lnb[64:65, :], AF.Exp, scale=-1.0
                )
                rb = ps_ms.tile([128, 512], F32, tag="mm", name=f"rb{p}_{qq}_{hi}")
                nc.tensor.matmul(
                    rb[0:64, :], oner[64:65, :], rec[64:65, :],
                    start=True, stop=True,
                )
                rbs = rcp.tile([64, 512], F32, tag="rbs", name=f"rbs{p}_{qq}_{hi}")
                nc.vector.tensor_copy(rbs[:, :], rb[0:64, :])
                dst = (
                    af[p][0:64, qq * 512:qe]
                    if hi == 0
                    else af1t[:, qq * 512:qe]
                )
                nc.vector.tensor_mul(dst, av[hi][0:64, 0:512], rbs[:, :])
            nc.sync.dma_start(
                af[p][64:128, qq * 512:qe], af1t[:, qq * 512:qe]
            )

        class Pacer:
            def __init__(self, fillers, total_steps):
                self.fillers = fillers
                self.total = max(1, total_steps)
                self.done = 0
                self.emitted = 0

            def step(self):
                self.done += 1
                want = min(
                    (len(self.fillers) * self.done) // self.total,
                    len(self.fillers),
                )
                while self.emitted < want:
                    self.fillers[self.emitted]()
                    self.emitted += 1

            def drain(self):
                while self.emitted < len(self.fillers):
                    self.fillers[self.emitted]()
                    self.emitted += 1

        # ---------------- main schedule ----------------------------------
        # slim preamble: only the chunks attention(0, qq0) needs; the rest
        # of pair 0's projections pace into its own attention windows.
        for kind, tc4 in PROJ_ORDER[:3]:
            for f in proj_unit(0, kind, tc4):
                f()

        for p in range(4):
            af1t = rcp.tile([64, N], BF16, tag="af1", name=f"af1_{p}")
            if p < 3:
                fillers = (make_proj_fillers(0, skip=3) if p == 0 else []) + \
                    make_proj_fillers(p + 1)
                # front-load pair 0's own remaining projections
                pacer = Pacer(fillers, 28 if p == 0 else 40)
                for qq in range(4):
                    attention(p, qq, af1t, pacer)
                pacer.drain()
            else:
                attention(p, 0, af1t, Pacer([], 4))
                for qq in range(1, 4):
                    u = []
                    for it in range(4 * (qq - 1), 4 * qq):
                        for oc in range(2):
                            u += p3_micros(it, oc)
                    pc = Pacer(u, 4 * qq + 4)
                    attention(p, qq, af1t, pc)
                    pc.drain()

        for it in range(12, 16):
            for oc in range(2):
                for f in p3_micros(it, oc):
                    f()

        for pool in (ps_ms, ps_av, ps_sc, osp, rcp, ewp, afp, vp, kqp, wp, xtp, cst):
            pool.release()

    _split_multi_waits(nc)
    return nc


_GRAPH = None


def _get_graph():
    global _GRAPH
    if _GRAPH is None:
        _GRAPH = build_graph()
    return _GRAPH


def kernel(x, mask, w_qkv, w_out, b_out):
    global LAST_RESULT
    x = np.asarray(x, dtype=np.float32)
    w_qkv = np.asarray(w_qkv, dtype=np.float32)
    w_out = np.asarray(w_out, dtype=np.float32)
    b_out = np.asarray(b_out, dtype=np.float32)

    nc = _get_graph()

    BF = ml_dtypes.bfloat16
    xT = [np.ascontiguousarray(x[b].T.astype(BF)) for b in range(B)]
    ii = np.arange(128)
    mask01 = np.where(ii[None, :] >= ii[:, None], 1.0, 0.0).astype(BF)
    ident = np.eye(128, dtype=np.float32).astype(BF)

    halves = []
    for h in range(2):
        o = 512 * h
        halves.append(
            {
                "w_q": np.ascontiguousarray(w_qkv[:, o:o + 512].astype(BF)),
                "w_k": np.ascontiguousarray(w_qkv[:, INNER + o:INNER + o + 512].astype(BF)),
                "w_v": np.ascontiguousarray(w_qkv[:, 2 * INNER + o:2 * INNER + o + 512].astype(BF)),
                "w_o": np.ascontiguousarray(w_out[o:o + 512, :].astype(BF)),
            }
        )

    in_maps = []
    for c in range(8):
        b = c // 2
        hv = halves[c % 2]
        in_maps.append(
            {
                "xT": xT[b],
                "w_q": hv["w_q"],
                "w_k": hv["w_k"],
                "w_v": hv["w_v"],
                "w_o": hv["w_o"],
                "mask01": mask01,
                "ident": ident,
            }
        )

    res = run_bass_kernel_spmd(nc, in_maps, list(range(8)))
    LAST_RESULT = res

    out = np.empty((B, N, DIM), dtype=np.float32)
    for b in range(B):
        out[b] = (
            res.results[2 * b]["out"].astype(np.float32)
            + res.results[2 * b + 1]["out"].astype(np.float32)
            + b_out[None, :]
        )
    return out
